# revision 3
# baseline (speedup 1.0000x reference)
"""Trainium2 Bass kernel: e3nn-style CudaTensorProduct (CG tensor product).

out[b, o] = sum_nnz cb * in1[b, i1] * in2[b, i2]

in1: [8192, 288] = 32 channels each of l1=0,1,2 (dims 1/3/5)
in2: [8192, 9]   = spherical harmonics l2=0..2
out: [8192, 2592]

Per core (batch slice of 1024, data parallel over 8 cores), per 128-row tile:
  p = (j, gm1) indexes the 81 product rows, q = (g, o3) the 81 output rows
  per channel.
    z[p, (ch, b)]   = in1[gm1(p), (ch, b)] * in2[j(p), b]
    out[(ch,b), q]  = sum_p z[p, (ch,b)] * T[p, q]        (flipped matmul:
                      lhsT = z 128-col chunk, rhs = T, out is batch-major so
                      the PSUM->SBUF convert covers only 81 q per channel
                      on 128 partitions)

  Channels 0-15 ("a-route"): in1 host-replicated to 81 rows, f16 in SBUF;
  the z multiply runs on GpSimd (ch 0-9) and DVE 2x (ch 10-15).
  Channels 16-31 ("c-route"): in1 arrives compact [9, cols]; the PE
  replicates it (lhsT = 0/1 matrix S) into f32 PSUM and DVE does the z
  multiply from PSUM at 1x.  This halves the in1 HBM traffic vs full host
  replication and keeps every engine at ~2.7-2.9us/tile:
    DMA 2.87 | DVE 2.84 | ACT 2.72 (converts) | Pool 2.63 | PE 2.05
  (PE runs at full p-state: the cost model reaches peak clock ~3us in.)
"""

from contextlib import ExitStack

import numpy as np

import concourse.bass as bass
import concourse.mybir as mybir
import concourse.tile as tile
from concourse import bacc
from concourse.bass_utils import run_bass_kernel_spmd

# ---- hardcoded problem geometry ----
B = 8192
DIM1 = 288
DIM2 = 9
CBH = 2592
NCORES = 8
BLOC = B // NCORES          # 1024 batch rows per core
PT = 128                    # batch rows per tile
NT = BLOC // PT             # 8 tiles per core
NCHAN = 32
NROW = 81                   # p = (j, gm1) product rows
NQ = 81                     # q output rows per channel
NA = 16                     # host-replicated channels (a-route)
NC = 16                     # PE-replicated channels (c-route)
NPOOL = 10                  # a-route channels multiplied on GpSimd
# mm2 rounds: channels per round and bank packing (6 per 2KB psum bank)
ROUNDS = ((0, 12), (12, 12), (24, 8))

# per group: (col offset in in1, 2*l1+1)
GRP = [(0, 1), (32, 3), (128, 5)]

F32 = mybir.dt.float32
F16 = mybir.dt.float16

_cache: dict = {}


# --------------------------------------------------------------------------
# Tables from the COO inputs
# --------------------------------------------------------------------------
def _build_tables(cb_vals, i1_idx, i2_idx, out_idx):
    """Build T [81, 81], perm2 [2592] from the COO triple.

    T[p, q] with p = j*9 + gm1: coefficient taking product row p to output
    row q.  perm2[ch*81 + q] = the out column for output row q of channel ch.
    """
    cb = np.asarray(cb_vals, np.float64)
    i1 = np.asarray(i1_idx, np.int64)
    i2 = np.asarray(i2_idx, np.int64)
    oo = np.asarray(out_idx, np.int64)

    g = np.where(i1 < 32, 0, np.where(i1 < 128, 1, 2))
    rel = i1 - np.array([0, 32, 128])[g]
    width = np.array([1, 3, 5])[g]
    c = rel // width
    m1 = rel % width
    gm1 = np.array([0, 1, 4])[g] + m1
    p = i2 * 9 + gm1                      # (j, gm1) ordering

    # distinct out columns per (g, c), sorted ascending -> rank within group
    qoff = {0: 0, 1: 9, 2: 36}
    d3 = {0: 9, 1: 27, 2: 45}
    ocols: dict = {}
    for gg, cc, o in zip(g, c, oo):
        ocols.setdefault((int(gg), int(cc)), set()).add(int(o))
    rank: dict = {}
    for (gg, cc), s in ocols.items():
        assert len(s) == d3[gg], f"group {gg} chan {cc}: {len(s)} != {d3[gg]}"
        for k, o in enumerate(sorted(s)):
            rank[(gg, cc, o)] = k

    T = np.zeros((NROW, NQ), np.float64)
    have = np.zeros((NROW, NQ), bool)
    colmap = -np.ones((NQ, NCHAN), np.int64)
    for n in range(len(cb)):
        gg, cc = int(g[n]), int(c[n])
        q = qoff[gg] + rank[(gg, cc, int(oo[n]))]
        colmap[q, cc] = oo[n]
        if have[p[n], q]:
            assert abs(T[p[n], q] - cb[n]) < 1e-5, "CG not channel-uniform"
        else:
            T[p[n], q] = cb[n]
            have[p[n], q] = True
    assert (colmap >= 0).all()
    perm2 = colmap.T.reshape(-1)          # ch-major device layout
    assert np.array_equal(np.sort(perm2), np.arange(CBH)), "not a permutation"

    S = np.zeros((9, NROW), np.float16)
    S[np.arange(NROW) % 9, np.arange(NROW)] = 1.0
    return T.astype(np.float16), S, perm2


# --------------------------------------------------------------------------
# Device kernel
# --------------------------------------------------------------------------
def _trace_module():
    nc = bacc.Bacc(trn_type="TRN2")
    # channels 0..15 host-replicated to all 81 (j, gm1) rows: [81, (t, ch, b)]
    in1b = nc.dram_tensor("in1b", [NROW, NT * NA * PT], F16, kind="ExternalInput")
    # channels 16..31 compact gm1-major: [9, (t, ch, b)]
    in1p = nc.dram_tensor("in1p", [9, NT * NC * PT], F16, kind="ExternalInput")
    # packed constants: [in2rep (BLOC) | T (81) | S rows on parts 0-8 (81)]
    cblob = nc.dram_tensor("cblob", [NROW, BLOC + NQ + NROW], F16, kind="ExternalInput")
    out16 = nc.dram_tensor("out16", [NT * PT, NCHAN * NQ], F16, kind="ExternalOutput")

    with tile.TileContext(nc) as tc, ExitStack() as ctx:
        _cg_body(ctx, tc, out16, in1b, in1p, cblob)
    nc.compile()
    return nc


def _cg_body(ctx, tc, out16, in1b, in1p, cblob):
    nc = tc.nc
    const = ctx.enter_context(tc.tile_pool(name="const", bufs=1))
    ibp = ctx.enter_context(tc.tile_pool(name="ibp", bufs=3))
    ipp = ctx.enter_context(tc.tile_pool(name="ipp", bufs=3))
    pap = ctx.enter_context(tc.tile_pool(name="pap", bufs=2, space="PSUM"))
    osp = ctx.enter_context(tc.tile_pool(name="osp", bufs=2, space="PSUM"))
    zp = ctx.enter_context(tc.tile_pool(name="zp", bufs=2))
    op = ctx.enter_context(tc.tile_pool(name="op", bufs=3))

    in1bv = in1b.ap().rearrange("p (t c b) -> p t c b", t=NT, c=NA)
    in1pv = in1p.ap().rearrange("g (t c b) -> g t c b", t=NT, c=NC)

    # one DMA for all constants (in2rep, T, S)
    sb_c = const.tile([NROW, BLOC + NQ + NROW], F16)
    nc.sync.dma_start(out=sb_c, in_=cblob.ap())
    sb_in2 = sb_c[:, 0:BLOC]
    lhs_s = sb_c[0:9, BLOC + NQ:BLOC + NQ + NROW]   # S [9, 81]
    rhs_t = sb_c[:, BLOC:BLOC + NQ]                  # T [81, 81]

    in1bt = [None] * NT
    in1pt = [None] * NT

    def _load(t):
        bt = ibp.tile([NROW, NA, PT], F16)
        nc.sync.dma_start(out=bt, in_=in1bv[:, t])
        in1bt[t] = bt
        pt_ = ipp.tile([9, NC, PT], F16)
        nc.sync.dma_start(out=pt_, in_=in1pv[:, t])
        in1pt[t] = pt_

    _load(0)
    _load(1)
    _load(2)

    mult = mybir.AluOpType.mult

    for t in range(NT):
        if t + 3 < NT:
            _load(t + 3)
        in2t = sb_in2[:, t * PT:(t + 1) * PT]

        # ---- a-route z: GpSimd ch 0..9, DVE ch 10..15 (all f16 SBUF, 2x)
        za = zp.tile([NROW, NA, PT], F16)
        i2a = in2t.unsqueeze(1).broadcast_to((NROW, NPOOL, PT))
        nc.gpsimd.tensor_tensor(
            out=za[:, 0:NPOOL], in0=in1bt[t][:, 0:NPOOL], in1=i2a, op=mult
        )
        i2d = in2t.unsqueeze(1).broadcast_to((NROW, NA - NPOOL, PT))
        nc.vector.tensor_tensor(
            out=za[:, NPOOL:NA], in0=in1bt[t][:, NPOOL:NA], in1=i2d, op=mult
        )

        # ---- c-route: PE replicate (2 fills of 8ch) + DVE z from PSUM
        zc = zp.tile([NROW, NC, PT], F16)
        for f in range(2):
            pa = pap.tile([NROW, 2, 512], F32)
            for k in range(2):
                nc.tensor.matmul(
                    pa[:, k],
                    lhsT=lhs_s,
                    rhs=in1pt[t][:, 8 * f + 4 * k:8 * f + 4 * k + 4, :]
                    .rearrange("g c b -> g (c b)"),
                    start=True,
                    stop=True,
                )
            i2c = in2t.unsqueeze(1).unsqueeze(1).broadcast_to((NROW, 2, 4, PT))
            nc.vector.tensor_tensor(
                out=zc[:, 8 * f:8 * f + 8].rearrange("p (k c) b -> p k c b", k=2),
                in0=pa[:].rearrange("p k (c b) -> p k c b", b=PT),
                in1=i2c,
                op=mult,
            )

        # ---- mm2 rounds + converts + store
        out_sb = op.tile([PT, NCHAN * NQ], F16)

        def _z(ch):
            if ch < NA:
                return za[:, ch, :]
            return zc[:, ch - NA, :]

        for r0, nch in ROUNDS:
            ob = osp.tile([PT, 2, 512], F32)
            pb = nch // 2    # channels per psum bank
            for k in range(nch):
                nc.tensor.matmul(
                    ob[:, k // pb, (k % pb) * NQ:(k % pb) * NQ + NQ],
                    lhsT=_z(r0 + k),
                    rhs=rhs_t,
                    start=True,
                    stop=True,
                )
            nc.scalar.copy(
                out=out_sb[:, r0 * NQ:(r0 + nch) * NQ]
                .rearrange("p (k x) -> p k x", k=2),
                in_=ob[:, :, 0:pb * NQ],
            )

        nc.sync.dma_start(
            out=out16.ap()[t * PT:(t + 1) * PT, :], in_=out_sb
        )


def _get_module():
    if "nc" not in _cache:
        _cache["nc"] = _trace_module()
    return _cache["nc"]


# --------------------------------------------------------------------------
# Host glue
# --------------------------------------------------------------------------
def _prep_in1(in1):
    """in1 [B, 288] -> per-core in1b [81, NT*16*128] f16 (ch 0-15 replicated)
    and in1p [9, NT*16*128] f16 (ch 16-31 compact)."""
    g0 = in1[:, 0:32].T[None]                                  # [1, 32, B]
    g1 = in1[:, 32:128].reshape(B, 32, 3).transpose(2, 1, 0)   # [3, 32, B]
    g2 = in1[:, 128:288].reshape(B, 32, 5).transpose(2, 1, 0)  # [5, 32, B]
    r = np.concatenate([g0, g1, g2], axis=0).astype(np.float16)  # [9, 32, B]
    rep = r[np.arange(NROW) % 9]            # [81, 32, B], p = (j, gm1)
    cores_b, cores_p = [], []
    for k in range(NCORES):
        rb = rep[:, 0:NA, k * BLOC:(k + 1) * BLOC].reshape(NROW, NA, NT, PT)
        rb = rb.transpose(0, 2, 1, 3).reshape(NROW, NT * NA * PT)
        cores_b.append(np.ascontiguousarray(rb))
        rp = r[:, NA:NCHAN, k * BLOC:(k + 1) * BLOC].reshape(9, NC, NT, PT)
        rp = rp.transpose(0, 2, 1, 3).reshape(9, NT * NC * PT)
        cores_p.append(np.ascontiguousarray(rp))
    return cores_b, cores_p


def _prep_const(in2, t_mat, s_mat):
    """Pack [in2rep | T | S] into one [81, BLOC+81+81] f16 blob per core."""
    rep = in2.T[np.arange(NROW) // 9].astype(np.float16)       # [81, B]
    blobs = []
    for k in range(NCORES):
        blob = np.zeros((NROW, BLOC + NQ + NROW), np.float16)
        blob[:, 0:BLOC] = rep[:, k * BLOC:(k + 1) * BLOC]
        blob[:, BLOC:BLOC + NQ] = t_mat
        blob[0:9, BLOC + NQ:] = s_mat
        blobs.append(blob)
    return blobs


def kernel(in1, in2, cb_vals, i1_idx, i2_idx, out_idx, **run_kwargs):
    in1 = np.asarray(in1, np.float32)
    in2 = np.asarray(in2, np.float32)
    assert in1.shape == (B, DIM1) and in2.shape == (B, DIM2)

    if "tables" not in _cache:
        _cache["tables"] = _build_tables(cb_vals, i1_idx, i2_idx, out_idx)
    t_mat, s_mat, perm2 = _cache["tables"]

    nc = _get_module()
    in1b_cores, in1p_cores = _prep_in1(in1)
    cblobs = _prep_const(in2, t_mat, s_mat)
    in_maps = [
        {"in1b": in1b_cores[k], "in1p": in1p_cores[k], "cblob": cblobs[k]}
        for k in range(NCORES)
    ]
    res = run_bass_kernel_spmd(nc, in_maps, core_ids=list(range(NCORES)), **run_kwargs)
    _cache["last_results"] = res

    out = np.empty((B, CBH), np.float32)
    for k in range(NCORES):
        od = np.asarray(res.results[k]["out16"]).astype(np.float32)
        out[k * BLOC:(k + 1) * BLOC, perm2] = od.reshape(BLOC, CBH)
    return out


# revision 38
# speedup vs baseline: 1.3309x; 1.3309x over previous
"""Trainium2 Bass kernel: e3nn-style CudaTensorProduct (CG tensor product).

out[b, o] = sum_nnz cb * in1[b, i1] * in2[b, i2]

in1: [8192, 288] = 32 channels each of l1=0,1,2 (dims 1/3/5)
in2: [8192, 9]   = spherical harmonics l2=0..2
out: [8192, 2592]

Per core (batch slice of 1024, data parallel over 8 cores), per 128-row tile:
  p = (j, gm1) indexes the 81 product rows, q = (g, o3) the 81 output rows
  per channel.
    z[p, (ch, b)]   = in1[gm1(p), (ch, b)] * in2[j(p), b]
    out[(ch,b), q]  = sum_p z[p, (ch,b)] * T[p, q]        (flipped matmul:
                      lhsT = z 128-col chunk, rhs = T, out is batch-major so
                      the PSUM->SBUF convert covers only 81 q per channel
                      on 128 partitions)

  Channels 0-15 ("a-route"): in1 host-replicated to 81 rows, f16 in SBUF;
  the z multiply runs on GpSimd (ch 0-9) and DVE 2x (ch 10-15).
  Channels 16-31 ("c-route"): in1 arrives compact [9, cols]; the PE
  replicates it (lhsT = 0/1 matrix S) into f32 PSUM and DVE does the z
  multiply from PSUM at 1x.  This halves the in1 HBM traffic vs full host
  replication and keeps every engine at ~2.7-2.9us/tile:
    DMA 2.87 | DVE 2.84 | ACT 2.72 (converts) | Pool 2.63 | PE 2.05
  (PE runs at full p-state: the cost model reaches peak clock ~3us in.)
"""

from contextlib import ExitStack

import numpy as np

import concourse.bass as bass
import concourse.mybir as mybir
import concourse.tile as tile
from concourse import bacc
from concourse.bass_utils import run_bass_kernel_spmd

# ---- hardcoded problem geometry ----
B = 8192
DIM1 = 288
DIM2 = 9
CBH = 2592
NCORES = 8
BLOC = B // NCORES          # 1024 batch rows per core
PT = 128                    # batch rows per tile
NT = BLOC // PT             # 8 tiles per core
NCHAN = 32
NROW = 81                   # p = (j, gm1) product rows
NQ = 81                     # q output rows per channel
NA = 16                     # host-replicated channels (a-route)
NC = 16                     # PE-replicated channels (c-route)
NPOOL = 10                  # a-route channels multiplied on GpSimd
NTX = 1                    # leading tiles with fully host-replicated in1
# mm2 rounds: channels per round and bank packing (6 per 2KB psum bank)
ROUNDS = ((0, 12), (12, 12), (24, 8))

# per group: (col offset in in1, 2*l1+1)
GRP = [(0, 1), (32, 3), (128, 5)]

F32 = mybir.dt.float32
F16 = mybir.dt.float16

_cache: dict = {}


# --------------------------------------------------------------------------
# Tables from the COO inputs
# --------------------------------------------------------------------------
def _build_tables(cb_vals, i1_idx, i2_idx, out_idx):
    """Build T [81, 81], perm2 [2592] from the COO triple.

    T[p, q] with p = j*9 + gm1: coefficient taking product row p to output
    row q.  perm2[ch*81 + q] = the out column for output row q of channel ch.
    """
    cb = np.asarray(cb_vals, np.float64)
    i1 = np.asarray(i1_idx, np.int64)
    i2 = np.asarray(i2_idx, np.int64)
    oo = np.asarray(out_idx, np.int64)

    g = np.where(i1 < 32, 0, np.where(i1 < 128, 1, 2))
    rel = i1 - np.array([0, 32, 128])[g]
    width = np.array([1, 3, 5])[g]
    c = rel // width
    m1 = rel % width
    gm1 = np.array([0, 1, 4])[g] + m1
    p = i2 * 9 + gm1                      # (j, gm1) ordering

    # distinct out columns per (g, c), sorted ascending -> rank within group
    qoff = {0: 0, 1: 9, 2: 36}
    d3 = {0: 9, 1: 27, 2: 45}
    ocols: dict = {}
    for gg, cc, o in zip(g, c, oo):
        ocols.setdefault((int(gg), int(cc)), set()).add(int(o))
    rank: dict = {}
    for (gg, cc), s in ocols.items():
        assert len(s) == d3[gg], f"group {gg} chan {cc}: {len(s)} != {d3[gg]}"
        for k, o in enumerate(sorted(s)):
            rank[(gg, cc, o)] = k

    T = np.zeros((NROW, NQ), np.float64)
    have = np.zeros((NROW, NQ), bool)
    colmap = -np.ones((NQ, NCHAN), np.int64)
    for n in range(len(cb)):
        gg, cc = int(g[n]), int(c[n])
        q = qoff[gg] + rank[(gg, cc, int(oo[n]))]
        colmap[q, cc] = oo[n]
        if have[p[n], q]:
            assert abs(T[p[n], q] - cb[n]) < 1e-5, "CG not channel-uniform"
        else:
            T[p[n], q] = cb[n]
            have[p[n], q] = True
    assert (colmap >= 0).all()
    perm2 = colmap.T.reshape(-1)          # ch-major device layout
    assert np.array_equal(np.sort(perm2), np.arange(CBH)), "not a permutation"

    S = np.zeros((9, NROW), np.float16)
    S[np.arange(NROW) % 9, np.arange(NROW)] = 1.0
    return T.astype(np.float16), S, perm2


# --------------------------------------------------------------------------
# Device kernel
# --------------------------------------------------------------------------
def _trace_module():
    nc = bacc.Bacc(trn_type="TRN2")
    # channels 0..15 host-replicated to all 81 (j, gm1) rows: [81, (t, ch, b)]
    in1b = nc.dram_tensor("in1b", [NROW, NT * NA * PT], F16, kind="ExternalInput")
    # channels 16..31 compact gm1-major: [9, (t, ch, b)]
    in1p = nc.dram_tensor("in1p", [9, NT * NC * PT], F16, kind="ExternalInput")
    # tiles 0-2 channels 16..31 host-replicated too: the PE runs at cold
    # p-state early on, so the first tiles skip their replication matmuls
    # (DMA has slack before the output stores start flowing)
    in1bx = nc.dram_tensor("in1bx", [NROW, NTX * NC * PT], F16, kind="ExternalInput")
    # packed constants: [in2rep (BLOC) | T (81) | S rows on parts 0-8 (81)]
    cblob = nc.dram_tensor("cblob", [NROW, BLOC + NQ + NROW], F16, kind="ExternalInput")
    out16 = nc.dram_tensor("out16", [NT * PT, NCHAN * NQ], F16, kind="ExternalOutput")

    with tile.TileContext(nc) as tc, ExitStack() as ctx:
        _cg_body(ctx, tc, out16, in1b, in1p, in1bx, cblob)
    nc.compile()
    return nc


def _cg_body(ctx, tc, out16, in1b, in1p, in1bx, cblob):
    nc = tc.nc
    const = ctx.enter_context(tc.tile_pool(name="const", bufs=1))
    ibp = ctx.enter_context(tc.tile_pool(name="ibp", bufs=4))
    ipp = ctx.enter_context(tc.tile_pool(name="ipp", bufs=4))
    pap = ctx.enter_context(tc.tile_pool(name="pap", bufs=2, space="PSUM"))
    osp = ctx.enter_context(tc.tile_pool(name="osp", bufs=2, space="PSUM"))
    zp = ctx.enter_context(tc.tile_pool(name="zp", bufs=4))
    op = ctx.enter_context(tc.tile_pool(name="op", bufs=3))

    in1bv = in1b.ap().rearrange("p (t c b) -> p t c b", t=NT, c=NA)
    in1pv = in1p.ap().rearrange("g (t c b) -> g t c b", t=NT, c=NC)

    in1bt = [None] * NT
    in1pt = [None] * NT

    def _load(t):
        bt = ibp.tile([NROW, NA, PT], F16)
        nc.sync.dma_start(out=bt, in_=in1bv[:, t])
        in1bt[t] = bt
        if t >= NTX:
            pt_ = ipp.tile([9, NC, PT], F16)
            nc.sync.dma_start(out=pt_, in_=in1pv[:, t])
            in1pt[t] = pt_

    # tile 0's z inputs first (critical path), then the rest.  The constant
    # blob is split so za(t0) waits only on in1b0 + the in2rep slab.
    bt0 = ibp.tile([NROW, NA, PT], F16)
    nc.sync.dma_start(out=bt0, in_=in1bv[:, 0])
    in1bt[0] = bt0
    sb_c = const.tile([NROW, BLOC + NQ + NROW], F16)
    nc.sync.dma_start(out=sb_c, in_=cblob.ap())
    sb_in2 = sb_c[:, 0:BLOC]
    lhs_s = sb_c[0:9, BLOC + NQ:BLOC + NQ + NROW]   # S [9, 81]
    rhs_t = sb_c[:, BLOC:BLOC + NQ]                  # T [81, 81]
    in1bxv = in1bx.ap().rearrange("p (t c b) -> p t c b", t=NTX, c=NC)
    bx0 = const.tile([NROW, NC, PT], F16)
    nc.sync.dma_start(out=bx0, in_=in1bxv[:, 0])
    bxt = [bx0]
    _load(1)
    if NTX > 1:
        bxr = const.tile([NROW, NTX - 1, NC, PT], F16)
        nc.sync.dma_start(out=bxr, in_=in1bxv[:, 1:NTX])
        bxt += [bxr[:, i] for i in range(NTX - 1)]
    _load(2)
    _load(3)

    mult = mybir.AluOpType.mult

    for t in range(NT):
        if t + 4 < NT:
            _load(t + 4)
        in2t = sb_in2[:, t * PT:(t + 1) * PT]

        # ---- a-route z: GpSimd ch 0..9, DVE ch 10..15 (all f16 SBUF, 2x).
        # Tile 0 runs fully on DVE (1.1us) — GpSimd would put 2.6us on the
        # cold-start critical path; it instead starts tile 1 early.
        za = zp.tile([NROW, NA, PT], F16)
        if t == 0:
            # all on DVE, in 4-channel chunks so mm2 can start sooner
            i2d = in2t.unsqueeze(1).broadcast_to((NROW, 4, PT))
            for s in range(0, NA, 4):
                nc.vector.tensor_tensor(
                    out=za[:, s:s + 4], in0=in1bt[t][:, s:s + 4], in1=i2d, op=mult
                )
        else:
            i2a = in2t.unsqueeze(1).broadcast_to((NROW, NPOOL, PT))
            nc.gpsimd.tensor_tensor(
                out=za[:, 0:NPOOL], in0=in1bt[t][:, 0:NPOOL], in1=i2a, op=mult
            )
            i2d = in2t.unsqueeze(1).broadcast_to((NROW, NA - NPOOL, PT))
            nc.vector.tensor_tensor(
                out=za[:, NPOOL:NA], in0=in1bt[t][:, NPOOL:NA], in1=i2d, op=mult
            )

        # ---- c-route: PE replicate (2 fills of 8ch) + DVE z from PSUM.
        # Tile 0: in1 for these channels came host-replicated (bx0), so z is
        # a plain SBUF multiply and the cold-p-state PE only runs mm2.
        zc = zp.tile([NROW, NC, PT], F16)
        if t < NTX:
            i2x = in2t.unsqueeze(1).broadcast_to((NROW, 4, PT))
            for s in range(0, NC, 4):
                nc.vector.tensor_tensor(
                    out=zc[:, s:s + 4], in0=bxt[t][:, s:s + 4], in1=i2x, op=mult
                )
        else:
            for f in range(2):
                pa = pap.tile([NROW, 2, 512], F32)
                for k in range(2):
                    nc.tensor.matmul(
                        pa[:, k],
                        lhsT=lhs_s,
                        rhs=in1pt[t][:, 8 * f + 4 * k:8 * f + 4 * k + 4, :]
                        .rearrange("g c b -> g (c b)"),
                        start=True,
                        stop=True,
                    )
                i2c = in2t.unsqueeze(1).unsqueeze(1).broadcast_to((NROW, 2, 4, PT))
                nc.vector.tensor_tensor(
                    out=zc[:, 8 * f:8 * f + 8].rearrange("p (k c) b -> p k c b", k=2),
                    in0=pa[:].rearrange("p k (c b) -> p k c b", b=PT),
                    in1=i2c,
                    op=mult,
                )

        # ---- mm2 rounds + converts + store
        out_sb = op.tile([PT, NCHAN * NQ], F16)

        def _z(ch):
            if ch < NA:
                return za[:, ch, :]
            return zc[:, ch - NA, :]

        for r0, nch in ROUNDS:
            ob = osp.tile([PT, 2, 512], F32)
            pb = nch // 2    # channels per psum bank
            for k in range(nch):
                nc.tensor.matmul(
                    ob[:, k // pb, (k % pb) * NQ:(k % pb) * NQ + NQ],
                    lhsT=_z(r0 + k),
                    rhs=rhs_t,
                    start=True,
                    stop=True,
                )
            nc.scalar.copy(
                out=out_sb[:, r0 * NQ:(r0 + nch) * NQ]
                .rearrange("p (k x) -> p k x", k=2),
                in_=ob[:, :, 0:pb * NQ],
            )
            # store each round as soon as its convert lands: the ~1.3us
            # HWDGE+DGE issue latency then overlaps the next round
            nc.sync.dma_start(
                out=out16.ap()[t * PT:(t + 1) * PT, r0 * NQ:(r0 + nch) * NQ],
                in_=out_sb[:, r0 * NQ:(r0 + nch) * NQ],
            )


def _get_module():
    if "nc" not in _cache:
        _cache["nc"] = _trace_module()
    return _cache["nc"]


# --------------------------------------------------------------------------
# Host glue
# --------------------------------------------------------------------------
def _prep_in1(in1):
    """in1 [B, 288] -> per-core in1b [81, NT*16*128] f16 (ch 0-15 replicated)
    and in1p [9, NT*16*128] f16 (ch 16-31 compact)."""
    g0 = in1[:, 0:32].T[None]                                  # [1, 32, B]
    g1 = in1[:, 32:128].reshape(B, 32, 3).transpose(2, 1, 0)   # [3, 32, B]
    g2 = in1[:, 128:288].reshape(B, 32, 5).transpose(2, 1, 0)  # [5, 32, B]
    r = np.concatenate([g0, g1, g2], axis=0).astype(np.float16)  # [9, 32, B]
    rep = r[np.arange(NROW) % 9]            # [81, 32, B], p = (j, gm1)
    cores_b, cores_p, cores_bx = [], [], []
    for k in range(NCORES):
        rb = rep[:, 0:NA, k * BLOC:(k + 1) * BLOC].reshape(NROW, NA, NT, PT)
        rb = rb.transpose(0, 2, 1, 3).reshape(NROW, NT * NA * PT)
        cores_b.append(np.ascontiguousarray(rb))
        rp = r[:, NA:NCHAN, k * BLOC:(k + 1) * BLOC].reshape(9, NC, NT, PT)
        rp = rp.transpose(0, 2, 1, 3).reshape(9, NT * NC * PT)
        cores_p.append(np.ascontiguousarray(rp))
        bx = rep[:, NA:NCHAN, k * BLOC:k * BLOC + NTX * PT]
        bx = bx.reshape(NROW, NC, NTX, PT).transpose(0, 2, 1, 3)
        cores_bx.append(np.ascontiguousarray(bx.reshape(NROW, NTX * NC * PT)))
    return cores_b, cores_p, cores_bx


def _prep_const(in2, t_mat, s_mat):
    """Pack [in2rep | T | S] into one [81, BLOC+81+81] f16 blob per core."""
    rep = in2.T[np.arange(NROW) // 9].astype(np.float16)       # [81, B]
    blobs = []
    for k in range(NCORES):
        blob = np.zeros((NROW, BLOC + NQ + NROW), np.float16)
        blob[:, 0:BLOC] = rep[:, k * BLOC:(k + 1) * BLOC]
        blob[:, BLOC:BLOC + NQ] = t_mat
        blob[0:9, BLOC + NQ:] = s_mat
        blobs.append(blob)
    return blobs


def kernel(in1, in2, cb_vals, i1_idx, i2_idx, out_idx, **run_kwargs):
    in1 = np.asarray(in1, np.float32)
    in2 = np.asarray(in2, np.float32)
    assert in1.shape == (B, DIM1) and in2.shape == (B, DIM2)

    if "tables" not in _cache:
        _cache["tables"] = _build_tables(cb_vals, i1_idx, i2_idx, out_idx)
    t_mat, s_mat, perm2 = _cache["tables"]

    nc = _get_module()
    in1b_cores, in1p_cores, in1bx_cores = _prep_in1(in1)
    cblobs = _prep_const(in2, t_mat, s_mat)
    in_maps = [
        {
            "in1b": in1b_cores[k],
            "in1p": in1p_cores[k],
            "in1bx": in1bx_cores[k],
            "cblob": cblobs[k],
        }
        for k in range(NCORES)
    ]
    res = run_bass_kernel_spmd(nc, in_maps, core_ids=list(range(NCORES)), **run_kwargs)
    _cache["last_results"] = res

    out = np.empty((B, CBH), np.float32)
    for k in range(NCORES):
        od = np.asarray(res.results[k]["out16"]).astype(np.float32)
        out[k * BLOC:(k + 1) * BLOC, perm2] = od.reshape(BLOC, CBH)
    return out
